# revision 19
# baseline (speedup 1.0000x reference)
"""Trainium2 Bass kernel for nn_MultiHeadAttention_80977313398935.

Causal multi-head attention, B=1 S=4096 D=512 H=8 HD=64, fp32 I/O.

Sharding (8 cores):
  - Queries: core c owns two 256-row chunks: A=[256c,256c+256), B=[3840-256c,4096-256c)
    (balanced causal work: every core sees ~4096 keys total across its chunks).
  - K/V projection: core c computes keys [512c,512c+512), then one AllGather
    shares K^T and V (with an embedded ones-column for the softmax denominator).
  - Attention in transposed layout: scores^T[k,q] = K^T_h^T-free matmuls with
    head-pair packing (two K=64 matmuls in row groups 0/64); exp on ACT with a
    per-group kill bias (host table) realizing causal truncation; diagonal
    tiles staged by dynamic-offset DMA + static triangle masks; A·V accumulated
    in PSUM [65,512] per head with denominator in row 64.
  - Output projection computed transposed (out^T = wo^T @ heads^T); the host
    transposes back and reassembles rows.

The program is SPMD (identical on all 8 cores); all per-core variation flows
through input data (tables of biases / offsets / mask tiles).
"""

import os
import sys

import numpy as np

for _p in ("/opt/trn_rl_repo", "/root/.axon_site/_ro/trn_rl_repo"):
    if os.path.isdir(_p) and _p not in sys.path:
        sys.path.insert(0, _p)

import concourse.bass as bass
import concourse.bacc as bacc
import concourse.mybir as mybir
import concourse.tile as tile

dt = mybir.dt
F32 = dt.float32
U32 = dt.uint32
AF = mybir.ActivationFunctionType

B, S, D, H = 1, 4096, 512, 8
HD = D // H          # 64
NCORES = 8
QW = 256             # q chunk width per chunk (2 chunks/core)
KU = 128             # keys per k-unit
SHARD = 512          # keys projected per core
NPAIR = 4            # head pairs
SCALE = 1.0 / float(np.sqrt(HD))
NEG = -1e9
VW = 65              # V cols per head incl. ones column
VROW = H * VW        # 520
NKU = S // KU        # 32 k-units total
GA_CAUSAL = 8        # chunk-A groups (2 units each) for causal variant
GB_CAUSAL = 16

MM_DT = dt.float32r  # matmul operand dtype (float32r = full-rate PE)


def build_bass(variant: str, mm_dt=MM_DT):
    """variant: 'causal' | 'zeros' | 'general'"""
    use_diag = variant == "causal"
    use_maskmul = variant == "general"
    ga = GA_CAUSAL if variant == "causal" else 16
    gb = GB_CAUSAL

    nc = bacc.Bacc(
        "TRN2", target_bir_lowering=False, debug=False, num_devices=NCORES,
        detect_race_conditions=False,
    )

    # ---- I/O ----
    qs_d = nc.dram_tensor("qs", [2 * QW, D], F32, kind="ExternalInput")
    vs_d = nc.dram_tensor("vs", [SHARD, D], F32, kind="ExternalInput")
    wq_d = nc.dram_tensor("wq", [D, D], mm_dt, kind="ExternalInput")
    wkv_d = nc.dram_tensor("wkv", [D, 2 * D], mm_dt, kind="ExternalInput")
    wo_d = nc.dram_tensor("wo", [D, D], mm_dt, kind="ExternalInput")
    wqb_d = nc.dram_tensor("wqb", [1, D], mm_dt, kind="ExternalInput")
    wkvb_d = nc.dram_tensor("wkvb", [1, 2 * D], mm_dt, kind="ExternalInput")
    wob_d = nc.dram_tensor("wob", [1, D], mm_dt, kind="ExternalInput")
    biasA_d = nc.dram_tensor("biasA", [128, 16], F32, kind="ExternalInput")
    biasB_d = nc.dram_tensor("biasB", [128, 16], F32, kind="ExternalInput")
    tri_d = nc.dram_tensor("trimask", [128, 2 * QW], mm_dt, kind="ExternalInput")
    offs_d = nc.dram_tensor("offs", [1, 8], U32, kind="ExternalInput")
    id_d = nc.dram_tensor("ident", [128, 128], F32, kind="ExternalInput")
    ones_d = nc.dram_tensor("onesrow", [1, 512], mm_dt, kind="ExternalInput")
    if use_maskmul:
        expm_d = nc.dram_tensor("expmT", [S, 2 * QW], mm_dt, kind="ExternalInput")
    out_d = nc.dram_tensor("outT", [D, 2 * QW], F32, kind="ExternalOutput")

    with tile.TileContext(nc) as tc:
        with (
            tc.tile_pool(name="const", bufs=1) as cpool,
            tc.tile_pool(name="big", bufs=1) as big,
            tc.tile_pool(name="dram", bufs=1, space="DRAM") as dpool,
        ):
            # ---- constants ----
            ident = cpool.tile([128, 128], F32)
            nc.sync.dma_start(ident[:, :], id_d[:, :])
            ones = cpool.tile([1, 512], mm_dt)
            nc.sync.dma_start(ones[:, :], ones_d[:, :])
            zbias = cpool.tile([128, 1], F32)
            nc.vector.memset(zbias[:, :], 0.0)
            biasA = cpool.tile([128, 16], F32)
            nc.sync.dma_start(biasA[:, :], biasA_d[:, :])
            biasB = cpool.tile([128, 16], F32)
            nc.sync.dma_start(biasB[:, :], biasB_d[:, :])
            tri = cpool.tile([128, 2 * QW], mm_dt)
            nc.sync.dma_start(tri[:, :], tri_d[:, :])
            wob = cpool.tile([1, D], mm_dt)
            nc.sync.dma_start(wob[:, :], wob_d[:, :])

            # ---- persistent attention-phase tiles ----
            QT = big.tile([128, NPAIR * 512], mm_dt)      # Q^T pair p at cols [512p, ...)
            wo_sb = big.tile([64, H * D], mm_dt)          # wo head-chunk h at cols [D*h, ...)
            headsT = [big.tile([64, 512], mm_dt, name=f"hT{h}") for h in range(H)]

            # DRAM internal tiles for the collective
            kv_shard = dpool.tile([2 * SHARD, VROW], mm_dt)
            kv_g = dpool.tile([NCORES * 2 * SHARD, VROW], mm_dt, addr_space="Shared")

            # ================= Phase 1: transposes + projections =================
            with (
                tc.tile_pool(name="p1", bufs=1) as p1,
                tc.tile_pool(name="pst", bufs=4, space="PSUM") as pst,
                tc.tile_pool(name="psp", bufs=2, space="PSUM") as psp,
            ):
                wqb = p1.tile([1, D], mm_dt)
                nc.sync.dma_start(wqb[:, :], wqb_d[:, :])
                wkvb = p1.tile([1, 2 * D], mm_dt)
                nc.sync.dma_start(wkvb[:, :], wkvb_d[:, :])
                qs = p1.tile([128, 4 * D], F32)   # row-tile r at cols [D*r, ...)
                nc.sync.dma_start(
                    qs[:, :].rearrange("p (r j) -> p r j", r=4),
                    qs_d[:, :].rearrange("(r p) j -> p r j", p=128),
                )
                vs = p1.tile([128, 4 * D], F32)
                nc.sync.dma_start(
                    vs[:, :].rearrange("p (r j) -> p r j", r=4),
                    vs_d[:, :].rearrange("(r p) j -> p r j", p=128),
                )
                wq = p1.tile([128, 4 * D], mm_dt)  # din-chunk ck at cols [D*ck, ...)
                nc.sync.dma_start(
                    wq[:, :].rearrange("p (c j) -> p c j", c=4),
                    wq_d[:, :].rearrange("(c p) j -> p c j", p=128),
                )
                wkv = p1.tile([128, 4 * 2 * D], mm_dt)
                nc.sync.dma_start(
                    wkv[:, :].rearrange("p (c j) -> p c j", c=4),
                    wkv_d[:, :].rearrange("(c p) j -> p c j", p=128),
                )
                nc.sync.dma_start(
                    wo_sb[:, :].rearrange("p (h j) -> p h j", h=H),
                    wo_d[:, :].rearrange("(h p) j -> p h j", p=64),
                )

                # transpose qs, vs -> qT, vT ([din, row] layout, din-chunk ck at cols [512ck,...))
                qT = p1.tile([128, 4 * 512], mm_dt)
                vT = p1.tile([128, 4 * 512], mm_dt)
                for src, dst in ((qs, qT), (vs, vT)):
                    for r in range(4):      # row tile
                        for d_ in range(4):  # din tile
                            pt = pst.tile([128, 128], F32, tag="tp")
                            nc.tensor.transpose(
                                pt[:, :], src[:, D * r + 128 * d_ : D * r + 128 * d_ + 128],
                                ident[:, :],
                            )
                            nc.vector.tensor_copy(
                                dst[:, 512 * d_ + 128 * r : 512 * d_ + 128 * r + 128],
                                pt[:, :],
                            )

                # Q^T projection: out pair-tile p = sum_ck wq[ck, tile p].T @ qT[ck]
                for p in range(NPAIR):
                    ps = psp.tile([128, 512], F32, tag="pj")
                    for ck in range(4):
                        nc.tensor.matmul(
                            ps[:, :],
                            wq[:, D * ck + 128 * p : D * ck + 128 * p + 128],
                            qT[:, 512 * ck : 512 * ck + 512],
                            start=(ck == 0), stop=False,
                        )
                    nc.tensor.matmul(
                        ps[:, :], wqb[:, 128 * p : 128 * p + 128], ones[:, :],
                        start=False, stop=True,
                    )
                    nc.vector.tensor_copy(QT[:, 512 * p : 512 * p + 512], ps[:, :])

                # K^T shard projection -> KTs [128, 4*SHARD]
                KTs = p1.tile([128, 4 * SHARD], mm_dt)
                for p in range(NPAIR):
                    ps = psp.tile([128, 512], F32, tag="pj")
                    for ck in range(4):
                        nc.tensor.matmul(
                            ps[:, :],
                            wkv[:, 2 * D * ck + 128 * p : 2 * D * ck + 128 * p + 128],
                            vT[:, 512 * ck : 512 * ck + 512],
                            start=(ck == 0), stop=False,
                        )
                    nc.tensor.matmul(
                        ps[:, :], wkvb[:, 128 * p : 128 * p + 128], ones[:, :],
                        start=False, stop=True,
                    )
                    nc.vector.tensor_copy(KTs[:, 512 * p : 512 * p + 512], ps[:, :])

                # V shard projection (natural [key, dv]) -> V1s [128, 4*VROW] with ones col
                V1s = p1.tile([128, 4 * VROW], mm_dt)
                for kt in range(4):
                    ps = psp.tile([128, 512], F32, tag="pj")
                    for ck in range(4):
                        nc.tensor.matmul(
                            ps[:, :],
                            vT[:, 512 * ck + 128 * kt : 512 * ck + 128 * kt + 128],
                            wkv[:, 2 * D * ck + D : 2 * D * ck + 2 * D],
                            start=(ck == 0), stop=False,
                        )
                    nc.tensor.matmul(
                        ps[:, :], ones[:, 0:128], wkvb[:, D : 2 * D],
                        start=False, stop=True,
                    )
                    nc.vector.tensor_copy(
                        V1s[:, VROW * kt : VROW * kt + VROW]
                        .rearrange("p (h j) -> p h j", h=H)[:, :, 0:HD],
                        ps[:, :],
                    )
                    nc.vector.tensor_scalar(
                        V1s[:, VROW * kt : VROW * kt + VROW]
                        .rearrange("p (h j) -> p h j", h=H)[:, :, HD : HD + 1],
                        ps[:, 0:H],
                        0.0,
                        1.0,
                        mybir.AluOpType.mult,
                        mybir.AluOpType.add,
                    )

                # shard -> DRAM (rows 0:512 K^T, 512:1024 V1)
                nc.sync.dma_start(
                    kv_shard[0:SHARD, 0:512].rearrange("(p r) j -> r p j", r=128),
                    KTs[:, :].rearrange("r (p j) -> r p j", p=4),
                )
                nc.sync.dma_start(
                    kv_shard[SHARD : 2 * SHARD, :].rearrange("(t r) j -> r t j", r=128),
                    V1s[:, :].rearrange("r (t j) -> r t j", t=4),
                )

            # ================= Phase 2: AllGather =================
            tc.strict_bb_all_engine_barrier()
            kvpool = tc.tile_pool(name="kv", bufs=1)
            kvp = kvpool.__enter__()
            KT = kvp.tile([128, NPAIR * S], mm_dt)   # K^T pair p at cols [S*p,S*p+S)
            V1 = kvp.tile([128, NKU * VROW], mm_dt)  # V+ones, unit u at cols [VROW*u, ...)
            nc.gpsimd.collective_compute(
                "AllGather",
                mybir.AluOpType.bypass,
                ins=[kv_shard[:, :].opt()],
                outs=[kv_g[:, :].opt()],
                replica_groups=[list(range(NCORES))],
            )

            # gathered -> SBUF
            for r in range(NCORES):
                nc.sync.dma_start(
                    KT[:, :].rearrange("i (p j) -> i p j", p=NPAIR)[
                        :, :, 512 * r : 512 * r + 512
                    ],
                    kv_g[1024 * r : 1024 * r + 512, 0:512].rearrange(
                        "(p i) j -> i p j", i=128
                    ),
                )
                nc.sync.dma_start(
                    V1[:, VROW * 4 * r : VROW * 4 * r + 4 * VROW].rearrange(
                        "i (t j) -> i t j", t=4
                    ),
                    kv_g[1024 * r + 512 : 1024 * r + 1024, :].rearrange(
                        "(t i) j -> i t j", i=128
                    ),
                )

            # ---- diagonal tile staging (causal variant) ----
            if use_diag:
                KTdg = {}
                V1dg = {}
                for ci, cname in enumerate("AB"):
                    KTdg[cname] = kvp.tile([128, 4 * 256], mm_dt, name=f"ktd{ci}")
                    V1dg[cname] = kvp.tile([128, 2 * VROW], mm_dt, name=f"v1d{ci}")
                if True:
                    with tc.tile_critical():
                        with (
                            nc.gpsimd.register("dgo") as r0,
                            nc.semaphore("dgsem") as dgsem,
                        ):
                            for ci, cname in enumerate("AB"):
                                nc.gpsimd.reg_load(r0, offs_d[0:1, 3 * ci : 3 * ci + 1])
                                ktrow = nc.gpsimd.snap(r0)
                                nc.gpsimd.reg_load(r0, offs_d[0:1, 3 * ci + 1 : 3 * ci + 2])
                                ktcol = nc.gpsimd.snap(r0)
                                nc.gpsimd.reg_load(r0, offs_d[0:1, 3 * ci + 2 : 3 * ci + 3])
                                vrow = nc.gpsimd.snap(r0)
                                nc.gpsimd.dma_start(
                                    KTdg[cname][:, :].rearrange("i (p j) -> i p j", p=4),
                                    kv_g[bass.ds(ktrow, 512), bass.ds(ktcol, 256)].rearrange(
                                        "(p i) j -> i p j", i=128
                                    ),
                                ).then_inc(dgsem, 16)
                                nc.gpsimd.dma_start(
                                    V1dg[cname][:, :].rearrange("i (u j) -> i u j", u=2),
                                    kv_g[bass.ds(vrow, 256), :].rearrange(
                                        "(u i) j -> i u j", i=128
                                    ),
                                ).then_inc(dgsem, 16)
                            nc.gpsimd.wait_ge(dgsem, 64)

            # ================= Phase 3: attention =================
            n_groups = {"A": ga, "B": gb}
            with (
                tc.tile_pool(name="acc", bufs=4, space="PSUM") as accp,
                tc.tile_pool(name="sc", bufs=4, space="PSUM") as scp,
                tc.tile_pool(name="ex", bufs=4) as exp_pool,
                tc.tile_pool(name="nrm", bufs=2) as nrm,
                tc.tile_pool(name="exm", bufs=2) as exmp,
            ):
                for wave in range(2):
                    heads = list(range(4 * wave, 4 * wave + 4))
                    acc = {h: accp.tile([VW, 512], F32, tag="acc", name=f"acc{h}") for h in heads}
                    for ci, cname in enumerate("AB"):
                        qoff = QW * ci
                        bias_t = biasA if cname == "A" else biasB
                        glist = [("reg", g) for g in range(n_groups[cname])]
                        if use_diag:
                            glist.append(("diag", 0))
                        for gkind, g in glist:
                            if use_maskmul:
                                exm = exmp.tile([128, 512], mm_dt, tag="exm")
                                nc.sync.dma_start(
                                    exm[:, :].rearrange("p (u j) -> p u j", u=2),
                                    expm_d[
                                        256 * g : 256 * g + 256, qoff : qoff + QW
                                    ].rearrange("(u p) j -> p u j", u=2),
                                )
                            for h in heads:
                                hp, hs = divmod(h, 2)  # pair index, side
                                sc = scp.tile([128, 512], F32, tag="sc")
                                qrhs = QT[
                                    64 * hs : 64 * hs + 64,
                                    512 * hp + qoff : 512 * hp + qoff + QW,
                                ]
                                for half in range(2):
                                    if gkind == "reg":
                                        u = 2 * g + half
                                        klhs = KT[
                                            64 * hs : 64 * hs + 64,
                                            S * hp + KU * u : S * hp + KU * u + KU,
                                        ]
                                    else:
                                        klhs = KTdg[cname][
                                            64 * hs : 64 * hs + 64,
                                            256 * hp + 128 * half : 256 * hp
                                            + 128 * half
                                            + 128,
                                        ]
                                    nc.tensor.matmul(
                                        sc[:, 256 * half : 256 * half + 256],
                                        klhs,
                                        qrhs,
                                        start=True,
                                        stop=(half == 1),
                                        tile_position=(64 * hs, 0),
                                        skip_group_check=True,
                                    )
                                ex = exp_pool.tile([128, 512], mm_dt, tag="ex")
                                bias_ap = (
                                    zbias[:, 0:1]
                                    if gkind == "diag"
                                    else bias_t[:, g : g + 1]
                                )
                                nc.scalar.activation(
                                    ex[:, :], sc[:, :], AF.Exp,
                                    bias=bias_ap, scale=SCALE,
                                )
                                if gkind == "diag":
                                    nc.vector.tensor_mul(ex[:, :], ex[:, :], tri[:, :])
                                if use_maskmul:
                                    nc.vector.tensor_mul(ex[:, :], ex[:, :], exm[:, :])
                                for half in range(2):
                                    if gkind == "reg":
                                        u = 2 * g + half
                                        vlhs = V1[
                                            :, VROW * u + VW * h : VROW * u + VW * h + VW
                                        ]
                                    else:
                                        vlhs = V1dg[cname][
                                            :,
                                            VROW * half + VW * h : VROW * half
                                            + VW * h
                                            + VW,
                                        ]
                                    first = gkind == "reg" and g == 0 and half == 0
                                    last = (
                                        (gkind == "diag" and half == 1)
                                        if use_diag
                                        else (
                                            gkind == "reg"
                                            and g == n_groups[cname] - 1
                                            and half == 1
                                        )
                                    )
                                    nc.tensor.matmul(
                                        acc[h][:, qoff : qoff + QW],
                                        vlhs,
                                        ex[:, 256 * half : 256 * half + 256],
                                        start=first,
                                        stop=last,
                                        skip_group_check=True,
                                    )
                    # normalize wave's heads
                    for h in heads:
                        rc = nrm.tile([1, 512], mm_dt, tag="rc")
                        with nc.allow_low_precision(reason="f32r is fp32-width"):
                            nc.vector.reciprocal(rc[:, :], acc[h][HD : HD + 1, :])
                        bc = scp.tile([64, 512], F32, tag="sc", name=f"bc{h}")
                        nc.tensor.matmul(
                            bc[:, :], ones[:, 0:64], rc[:, :], start=True, stop=True,
                        )
                        bcs = nrm.tile([64, 512], F32, tag="bcs", name=f"bcs{h}")
                        nc.vector.tensor_copy(bcs[:, :], bc[:, :])
                        nc.vector.tensor_mul(
                            headsT[h][:, :], acc[h][0:HD, :], bcs[:, :]
                        )

            # ================= Phase 4: output projection (transposed) =========
            with (
                tc.tile_pool(name="po", bufs=2, space="PSUM") as pop,
                tc.tile_pool(name="ot", bufs=2) as otp,
            ):
                for t in range(4):
                    po = pop.tile([128, 512], F32, tag="po")
                    for h in range(H):
                        nc.tensor.matmul(
                            po[:, :],
                            wo_sb[:, D * h + 128 * t : D * h + 128 * t + 128],
                            headsT[h][:, :],
                            start=(h == 0), stop=False,
                        )
                    nc.tensor.matmul(
                        po[:, :], wob[:, 128 * t : 128 * t + 128], ones[:, :],
                        start=False, stop=True,
                    )
                    ot = otp.tile([128, 512], F32, tag="ot")
                    nc.vector.tensor_copy(ot[:, :], po[:, :])
                    nc.sync.dma_start(out_d[128 * t : 128 * t + 128, :], ot[:, :])
            kvpool.__exit__(None, None, None)

    nc.compile()
    return nc


# ---------------------------------------------------------------------------
# Host-side sharding / assembly
# ---------------------------------------------------------------------------

def classify_mask(mask: np.ndarray) -> str:
    m = mask.reshape(S, S)
    r = np.arange(S)
    valid = r[None, :] <= r[:, None]
    if np.all(m[valid] == 0.0):
        if np.all(m[~valid] <= -1e8):
            return "causal"
    if np.all(m == 0.0):
        return "zeros"
    return "general"


def make_inputs(variant, c, query, value, mask, wq_k, wq_b, wkv_k, wkv_b, wo_k, wo_b):
    q = query.reshape(S, D)
    v = value.reshape(S, D)
    qa0 = QW * c
    qb0 = S - QW * (c + 1)
    qs = np.concatenate([q[qa0 : qa0 + QW], q[qb0 : qb0 + QW]], axis=0)
    vs = v[SHARD * c : SHARD * (c + 1)]

    f32 = np.float32
    biasA = np.zeros((128, 16), f32)
    biasB = np.zeros((128, 16), f32)
    tri = np.zeros((128, 2 * QW), f32)
    offs = np.zeros((1, 8), np.uint32)
    if variant == "causal":
        for g in range(16):
            if g >= c:
                biasA[:, g] = NEG
            if g > 14 - c:
                biasB[:, g] = NEG
        jj = np.arange(QW)[None, :]
        pp = np.arange(128)[:, None]
        tri[:, 0:QW] = (pp <= jj).astype(f32)
        tri[:, QW:] = (pp + 128 <= jj).astype(f32)
        uA = 2 * c
        uB = (S - QW * (c + 1)) // KU  # first diag unit of chunk B = 30-2c
        offs[0, 0:3] = (1024 * (uA // 4), 128 * (uA % 4),
                        1024 * (uA // 4) + 512 + 128 * (uA % 4))
        offs[0, 3:6] = (1024 * (uB // 4), 128 * (uB % 4),
                        1024 * (uB // 4) + 512 + 128 * (uB % 4))

    ins = {
        "qs": np.ascontiguousarray(qs, f32),
        "vs": np.ascontiguousarray(vs, f32),
        "wq": np.ascontiguousarray(wq_k, f32),
        "wkv": np.ascontiguousarray(wkv_k, f32),
        "wo": np.ascontiguousarray(wo_k, f32),
        "wqb": np.ascontiguousarray(wq_b.reshape(1, D), f32),
        "wkvb": np.ascontiguousarray(wkv_b.reshape(1, 2 * D), f32),
        "wob": np.ascontiguousarray(wo_b.reshape(1, D), f32),
        "biasA": biasA,
        "biasB": biasB,
        "trimask": tri,
        "offs": offs,
        "ident": np.eye(128, dtype=f32),
        "onesrow": np.ones((1, 512), f32),
    }
    if variant == "general":
        m = mask.reshape(S, S)
        rows = np.concatenate(
            [np.arange(qa0, qa0 + QW), np.arange(qb0, qb0 + QW)]
        )
        ins["expmT"] = np.ascontiguousarray(
            np.exp(np.minimum(m[rows, :], 80.0)).T, f32
        )
    return ins


def assemble(results):
    full = np.empty((S, D), np.float32)
    for c in range(NCORES):
        o = results[c]["outT"].T  # [512 q, 512 d]
        full[QW * c : QW * c + QW] = o[0:QW]
        full[S - QW * (c + 1) : S - QW * c] = o[QW:]
    return full.reshape(B, S, D)


_cache = {}
last_results = None


def kernel(query, value, mask, wq_k, wq_b, wkv_k, wkv_b, wo_k, wo_b, **run_kwargs):
    global last_results
    from concourse.bass_utils import run_bass_kernel_spmd

    variant = classify_mask(np.asarray(mask))
    if variant not in _cache:
        _cache[variant] = build_bass(variant)
    nc = _cache[variant]
    in_maps = [
        make_inputs(variant, c, query, value, mask,
                    wq_k, wq_b, wkv_k, wkv_b, wo_k, wo_b)
        for c in range(NCORES)
    ]
    res = run_bass_kernel_spmd(nc, in_maps, core_ids=list(range(NCORES)), **run_kwargs)
    last_results = res
    return assemble(res.results)


# revision 22
# speedup vs baseline: 433.4230x; 433.4230x over previous
"""Trainium2 Bass kernel for nn_MultiHeadAttention_80977313398935.

Causal multi-head attention, B=1 S=4096 D=512 H=8 HD=64, fp32 I/O.

Sharding (8 cores):
  - Queries: core c owns two 256-row chunks: A=[256c,256c+256), B=[3840-256c,4096-256c)
    (balanced causal work: every core sees ~4096 keys total across its chunks).
  - K/V projection: core c computes keys [512c,512c+512), then one AllGather
    shares K^T and V (with an embedded ones-column for the softmax denominator).
  - Attention in transposed layout: scores^T[k,q] = K^T_h^T-free matmuls with
    head-pair packing (two K=64 matmuls in row groups 0/64); exp on ACT with a
    per-group kill bias (host table) realizing causal truncation; diagonal
    tiles staged by dynamic-offset DMA + static triangle masks; A·V accumulated
    in PSUM [65,512] per head with denominator in row 64.
  - Output projection computed transposed (out^T = wo^T @ heads^T); the host
    transposes back and reassembles rows.

The program is SPMD (identical on all 8 cores); all per-core variation flows
through input data (tables of biases / offsets / mask tiles).
"""

import os
import sys

import numpy as np

for _p in ("/opt/trn_rl_repo", "/root/.axon_site/_ro/trn_rl_repo"):
    if os.path.isdir(_p) and _p not in sys.path:
        sys.path.insert(0, _p)

import concourse.bass as bass
import concourse.bacc as bacc
import concourse.mybir as mybir
import concourse.tile as tile

dt = mybir.dt
F32 = dt.float32
U32 = dt.uint32
AF = mybir.ActivationFunctionType

B, S, D, H = 1, 4096, 512, 8
HD = D // H          # 64
NCORES = 8
QW = 256             # q chunk width per chunk (2 chunks/core)
KU = 128             # keys per k-unit
SHARD = 512          # keys projected per core
NPAIR = 4            # head pairs
SCALE = 1.0 / float(np.sqrt(HD))
NEG = -1e9
VW = 65              # V cols per head incl. ones column
VROW = H * VW        # 520
NKU = S // KU        # 32 k-units total
GA_CAUSAL = 8        # chunk-A groups (2 units each) for causal variant
GB_CAUSAL = 16

MM_DT = dt.float32r  # matmul operand dtype (float32r = full-rate PE)


def build_bass(variant: str, mm_dt=MM_DT, collective=True):
    """variant: 'causal' | 'zeros' | 'general'"""
    use_diag = variant == "causal"
    use_maskmul = variant == "general"
    ga = GA_CAUSAL if variant == "causal" else 16
    gb = GB_CAUSAL

    nc = bacc.Bacc(
        "TRN2", target_bir_lowering=False, debug=False,
        num_devices=NCORES if collective else 1,
        detect_race_conditions=False,
    )

    # ---- I/O ----
    qs_d = nc.dram_tensor("qs", [2 * QW, D], F32, kind="ExternalInput")
    vs_d = nc.dram_tensor("vs", [SHARD, D], F32, kind="ExternalInput")
    wq_d = nc.dram_tensor("wq", [D, D], mm_dt, kind="ExternalInput")
    wkv_d = nc.dram_tensor("wkv", [D, 2 * D], mm_dt, kind="ExternalInput")
    wo_d = nc.dram_tensor("wo", [D, D], mm_dt, kind="ExternalInput")
    wqb_d = nc.dram_tensor("wqb", [1, D], mm_dt, kind="ExternalInput")
    wkvb_d = nc.dram_tensor("wkvb", [1, 2 * D], mm_dt, kind="ExternalInput")
    wob_d = nc.dram_tensor("wob", [1, D], mm_dt, kind="ExternalInput")
    biasA_d = nc.dram_tensor("biasA", [128, 16], F32, kind="ExternalInput")
    biasB_d = nc.dram_tensor("biasB", [128, 16], F32, kind="ExternalInput")
    tri_d = nc.dram_tensor("trimask", [128, 2 * QW], mm_dt, kind="ExternalInput")
    offs_d = nc.dram_tensor("offs", [1, 8], U32, kind="ExternalInput")
    id_d = nc.dram_tensor("ident", [128, 128], F32, kind="ExternalInput")
    ones_d = nc.dram_tensor("onesrow", [1, 512], mm_dt, kind="ExternalInput")
    if use_maskmul:
        expm_d = nc.dram_tensor("expmT", [S, 2 * QW], mm_dt, kind="ExternalInput")
    out_d = nc.dram_tensor("outT", [D, 2 * QW], F32, kind="ExternalOutput")

    with tile.TileContext(nc) as tc:
        with (
            tc.tile_pool(name="const", bufs=1) as cpool,
            tc.tile_pool(name="big", bufs=1) as big,
            tc.tile_pool(name="dram", bufs=1, space="DRAM") as dpool,
        ):
            # ---- constants ----
            ident = cpool.tile([128, 128], F32)
            nc.sync.dma_start(ident[:, :], id_d[:, :])
            ones = cpool.tile([1, 512], mm_dt)
            nc.sync.dma_start(ones[:, :], ones_d[:, :])
            zbias = cpool.tile([128, 1], F32)
            nc.vector.memset(zbias[:, :], 0.0)
            biasA = cpool.tile([128, 16], F32)
            nc.sync.dma_start(biasA[:, :], biasA_d[:, :])
            biasB = cpool.tile([128, 16], F32)
            nc.sync.dma_start(biasB[:, :], biasB_d[:, :])
            tri = cpool.tile([128, 2 * QW], mm_dt)
            nc.sync.dma_start(tri[:, :], tri_d[:, :])
            wob = cpool.tile([1, D], mm_dt)
            nc.sync.dma_start(wob[:, :], wob_d[:, :])

            # ---- persistent attention-phase tiles ----
            QT = big.tile([128, NPAIR * 512], mm_dt)      # Q^T pair p at cols [512p, ...)
            wo_sb = big.tile([64, H * D], mm_dt)          # wo head-chunk h at cols [D*h, ...)
            headsT = [big.tile([64, 512], mm_dt, name=f"hT{h}") for h in range(H)]

            # DRAM internal tiles for the collective
            kv_shard = dpool.tile([2 * SHARD, VROW], mm_dt)
            kv_g = dpool.tile([NCORES * 2 * SHARD, VROW], mm_dt, addr_space="Shared")

            # ================= Phase 1: transposes + projections =================
            with (
                tc.tile_pool(name="p1", bufs=1) as p1,
                tc.tile_pool(name="pst", bufs=4, space="PSUM") as pst,
                tc.tile_pool(name="psp", bufs=2, space="PSUM") as psp,
            ):
                wqb = p1.tile([1, D], mm_dt)
                nc.sync.dma_start(wqb[:, :], wqb_d[:, :])
                wkvb = p1.tile([1, 2 * D], mm_dt)
                nc.sync.dma_start(wkvb[:, :], wkvb_d[:, :])
                qs = p1.tile([128, 4 * D], F32)   # row-tile r at cols [D*r, ...)
                nc.sync.dma_start(
                    qs[:, :].rearrange("p (r j) -> p r j", r=4),
                    qs_d[:, :].rearrange("(r p) j -> p r j", p=128),
                )
                vs = p1.tile([128, 4 * D], F32)
                nc.sync.dma_start(
                    vs[:, :].rearrange("p (r j) -> p r j", r=4),
                    vs_d[:, :].rearrange("(r p) j -> p r j", p=128),
                )
                wq = p1.tile([128, 4 * D], mm_dt)  # din-chunk ck at cols [D*ck, ...)
                nc.sync.dma_start(
                    wq[:, :].rearrange("p (c j) -> p c j", c=4),
                    wq_d[:, :].rearrange("(c p) j -> p c j", p=128),
                )
                wkv = p1.tile([128, 4 * 2 * D], mm_dt)
                nc.sync.dma_start(
                    wkv[:, :].rearrange("p (c j) -> p c j", c=4),
                    wkv_d[:, :].rearrange("(c p) j -> p c j", p=128),
                )
                nc.sync.dma_start(
                    wo_sb[:, :].rearrange("p (h j) -> p h j", h=H),
                    wo_d[:, :].rearrange("(h p) j -> p h j", p=64),
                )

                # transpose qs, vs -> qT, vT ([din, row] layout, din-chunk ck at cols [512ck,...))
                qT = p1.tile([128, 4 * 512], mm_dt)
                vT = p1.tile([128, 4 * 512], mm_dt)
                for src, dst in ((qs, qT), (vs, vT)):
                    for r in range(4):      # row tile
                        for d_ in range(4):  # din tile
                            pt = pst.tile([128, 128], F32, tag="tp")
                            nc.tensor.transpose(
                                pt[:, :], src[:, D * r + 128 * d_ : D * r + 128 * d_ + 128],
                                ident[:, :],
                            )
                            nc.vector.tensor_copy(
                                dst[:, 512 * d_ + 128 * r : 512 * d_ + 128 * r + 128],
                                pt[:, :],
                            )

                # Q^T projection: out pair-tile p = sum_ck wq[ck, tile p].T @ qT[ck]
                for p in range(NPAIR):
                    ps = psp.tile([128, 512], F32, tag="pj")
                    for ck in range(4):
                        nc.tensor.matmul(
                            ps[:, :],
                            wq[:, D * ck + 128 * p : D * ck + 128 * p + 128],
                            qT[:, 512 * ck : 512 * ck + 512],
                            start=(ck == 0), stop=False,
                        )
                    nc.tensor.matmul(
                        ps[:, :], wqb[:, 128 * p : 128 * p + 128], ones[:, :],
                        start=False, stop=True,
                    )
                    nc.vector.tensor_copy(QT[:, 512 * p : 512 * p + 512], ps[:, :])

                # K^T shard projection -> KTs [128, 4*SHARD]
                KTs = p1.tile([128, 4 * SHARD], mm_dt)
                for p in range(NPAIR):
                    ps = psp.tile([128, 512], F32, tag="pj")
                    for ck in range(4):
                        nc.tensor.matmul(
                            ps[:, :],
                            wkv[:, 2 * D * ck + 128 * p : 2 * D * ck + 128 * p + 128],
                            vT[:, 512 * ck : 512 * ck + 512],
                            start=(ck == 0), stop=False,
                        )
                    nc.tensor.matmul(
                        ps[:, :], wkvb[:, 128 * p : 128 * p + 128], ones[:, :],
                        start=False, stop=True,
                    )
                    nc.vector.tensor_copy(KTs[:, 512 * p : 512 * p + 512], ps[:, :])

                # V shard projection (natural [key, dv]) -> V1s [128, 4*VROW] with ones col
                V1s = p1.tile([128, 4 * VROW], mm_dt)
                for kt in range(4):
                    ps = psp.tile([128, 512], F32, tag="pj")
                    for ck in range(4):
                        nc.tensor.matmul(
                            ps[:, :],
                            vT[:, 512 * ck + 128 * kt : 512 * ck + 128 * kt + 128],
                            wkv[:, 2 * D * ck + D : 2 * D * ck + 2 * D],
                            start=(ck == 0), stop=False,
                        )
                    nc.tensor.matmul(
                        ps[:, :], ones[:, 0:128], wkvb[:, D : 2 * D],
                        start=False, stop=True,
                    )
                    nc.vector.tensor_copy(
                        V1s[:, VROW * kt : VROW * kt + VROW]
                        .rearrange("p (h j) -> p h j", h=H)[:, :, 0:HD],
                        ps[:, :],
                    )
                    nc.vector.tensor_scalar(
                        V1s[:, VROW * kt : VROW * kt + VROW]
                        .rearrange("p (h j) -> p h j", h=H)[:, :, HD : HD + 1],
                        ps[:, 0:H],
                        0.0,
                        1.0,
                        mybir.AluOpType.mult,
                        mybir.AluOpType.add,
                    )

                # shard -> DRAM (rows 0:512 K^T, 512:1024 V1)
                nc.sync.dma_start(
                    kv_shard[0:SHARD, 0:512].rearrange("(p r) j -> r p j", r=128),
                    KTs[:, :].rearrange("r (p j) -> r p j", p=4),
                )
                nc.sync.dma_start(
                    kv_shard[SHARD : 2 * SHARD, :].rearrange("(t r) j -> r t j", r=128),
                    V1s[:, :].rearrange("r (t j) -> r t j", t=4),
                )

            # ================= Phase 2: AllGather =================
            tc.strict_bb_all_engine_barrier()
            kvpool = tc.tile_pool(name="kv", bufs=1)
            kvp = kvpool.__enter__()
            KT = kvp.tile([128, NPAIR * S], mm_dt)   # K^T pair p at cols [S*p,S*p+S)
            V1 = kvp.tile([128, NKU * VROW], mm_dt)  # V+ones, unit u at cols [VROW*u, ...)
            if collective:
                nc.gpsimd.collective_compute(
                    "AllGather",
                    mybir.AluOpType.bypass,
                    ins=[kv_shard[:, :].opt()],
                    outs=[kv_g[:, :].opt()],
                    replica_groups=[list(range(NCORES))],
                )
            else:
                # timing-model stand-in: move the shard into block 0
                nc.sync.dma_start(kv_g[0 : 2 * SHARD, :], kv_shard[:, :])

            # gathered -> SBUF
            for r in range(NCORES):
                nc.sync.dma_start(
                    KT[:, :].rearrange("i (p j) -> i p j", p=NPAIR)[
                        :, :, 512 * r : 512 * r + 512
                    ],
                    kv_g[1024 * r : 1024 * r + 512, 0:512].rearrange(
                        "(p i) j -> i p j", i=128
                    ),
                )
                nc.sync.dma_start(
                    V1[:, VROW * 4 * r : VROW * 4 * r + 4 * VROW].rearrange(
                        "i (t j) -> i t j", t=4
                    ),
                    kv_g[1024 * r + 512 : 1024 * r + 1024, :].rearrange(
                        "(t i) j -> i t j", i=128
                    ),
                )

            # ---- diagonal tile staging (causal variant) ----
            if use_diag:
                KTdg = {}
                V1dg = {}
                for ci, cname in enumerate("AB"):
                    KTdg[cname] = kvp.tile([128, 4 * 256], mm_dt, name=f"ktd{ci}")
                    V1dg[cname] = kvp.tile([128, 2 * VROW], mm_dt, name=f"v1d{ci}")
                if True:
                    with tc.tile_critical():
                        with (
                            nc.gpsimd.register("dgo") as r0,
                            nc.semaphore("dgsem") as dgsem,
                        ):
                            for ci, cname in enumerate("AB"):
                                nc.gpsimd.reg_load(r0, offs_d[0:1, 3 * ci : 3 * ci + 1])
                                ktrow = nc.gpsimd.snap(r0)
                                nc.gpsimd.reg_load(r0, offs_d[0:1, 3 * ci + 1 : 3 * ci + 2])
                                ktcol = nc.gpsimd.snap(r0)
                                nc.gpsimd.reg_load(r0, offs_d[0:1, 3 * ci + 2 : 3 * ci + 3])
                                vrow = nc.gpsimd.snap(r0)
                                nc.gpsimd.dma_start(
                                    KTdg[cname][:, :].rearrange("i (p j) -> i p j", p=4),
                                    kv_g[bass.ds(ktrow, 512), bass.ds(ktcol, 256)].rearrange(
                                        "(p i) j -> i p j", i=128
                                    ),
                                ).then_inc(dgsem, 16)
                                nc.gpsimd.dma_start(
                                    V1dg[cname][:, :].rearrange("i (u j) -> i u j", u=2),
                                    kv_g[bass.ds(vrow, 256), :].rearrange(
                                        "(u i) j -> i u j", i=128
                                    ),
                                ).then_inc(dgsem, 16)
                            nc.gpsimd.wait_ge(dgsem, 64)

            # ================= Phase 3: attention =================
            n_groups = {"A": ga, "B": gb}
            with (
                tc.tile_pool(name="acc", bufs=4, space="PSUM") as accp,
                tc.tile_pool(name="sc", bufs=4, space="PSUM") as scp,
                tc.tile_pool(name="ex", bufs=4) as exp_pool,
                tc.tile_pool(name="nrm", bufs=2) as nrm,
                tc.tile_pool(name="exm", bufs=2) as exmp,
            ):
                for wave in range(2):
                    heads = list(range(4 * wave, 4 * wave + 4))
                    acc = {h: accp.tile([VW, 512], F32, tag="acc", name=f"acc{h}") for h in heads}
                    for ci, cname in enumerate("AB"):
                        qoff = QW * ci
                        bias_t = biasA if cname == "A" else biasB
                        glist = [("reg", g) for g in range(n_groups[cname])]
                        if use_diag:
                            glist.append(("diag", 0))
                        for gkind, g in glist:
                            if use_maskmul:
                                exm = exmp.tile([128, 512], mm_dt, tag="exm")
                                nc.sync.dma_start(
                                    exm[:, :].rearrange("p (u j) -> p u j", u=2),
                                    expm_d[
                                        256 * g : 256 * g + 256, qoff : qoff + QW
                                    ].rearrange("(u p) j -> p u j", u=2),
                                )
                            for h in heads:
                                hp, hs = divmod(h, 2)  # pair index, side
                                sc = scp.tile([128, 512], F32, tag="sc")
                                qrhs = QT[
                                    64 * hs : 64 * hs + 64,
                                    512 * hp + qoff : 512 * hp + qoff + QW,
                                ]
                                for half in range(2):
                                    if gkind == "reg":
                                        u = 2 * g + half
                                        klhs = KT[
                                            64 * hs : 64 * hs + 64,
                                            S * hp + KU * u : S * hp + KU * u + KU,
                                        ]
                                    else:
                                        klhs = KTdg[cname][
                                            64 * hs : 64 * hs + 64,
                                            256 * hp + 128 * half : 256 * hp
                                            + 128 * half
                                            + 128,
                                        ]
                                    nc.tensor.matmul(
                                        sc[:, 256 * half : 256 * half + 256],
                                        klhs,
                                        qrhs,
                                        start=True,
                                        stop=(half == 1),
                                        tile_position=(64 * hs, 0),
                                        skip_group_check=True,
                                    )
                                ex = exp_pool.tile([128, 512], mm_dt, tag="ex")
                                bias_ap = (
                                    zbias[:, 0:1]
                                    if gkind == "diag"
                                    else bias_t[:, g : g + 1]
                                )
                                nc.scalar.activation(
                                    ex[:, :], sc[:, :], AF.Exp,
                                    bias=bias_ap, scale=SCALE,
                                )
                                if gkind == "diag":
                                    nc.vector.tensor_mul(ex[:, :], ex[:, :], tri[:, :])
                                if use_maskmul:
                                    nc.vector.tensor_mul(ex[:, :], ex[:, :], exm[:, :])
                                for half in range(2):
                                    if gkind == "reg":
                                        u = 2 * g + half
                                        vlhs = V1[
                                            :, VROW * u + VW * h : VROW * u + VW * h + VW
                                        ]
                                    else:
                                        vlhs = V1dg[cname][
                                            :,
                                            VROW * half + VW * h : VROW * half
                                            + VW * h
                                            + VW,
                                        ]
                                    first = gkind == "reg" and g == 0 and half == 0
                                    last = (
                                        (gkind == "diag" and half == 1)
                                        if use_diag
                                        else (
                                            gkind == "reg"
                                            and g == n_groups[cname] - 1
                                            and half == 1
                                        )
                                    )
                                    nc.tensor.matmul(
                                        acc[h][:, qoff : qoff + QW],
                                        vlhs,
                                        ex[:, 256 * half : 256 * half + 256],
                                        start=first,
                                        stop=last,
                                        skip_group_check=True,
                                    )
                    # normalize wave's heads
                    for h in heads:
                        rc = nrm.tile([1, 512], mm_dt, tag="rc")
                        with nc.allow_low_precision(reason="f32r is fp32-width"):
                            nc.vector.reciprocal(rc[:, :], acc[h][HD : HD + 1, :])
                        bc = scp.tile([64, 512], F32, tag="sc", name=f"bc{h}")
                        nc.tensor.matmul(
                            bc[:, :], ones[:, 0:64], rc[:, :], start=True, stop=True,
                        )
                        bcs = nrm.tile([64, 512], F32, tag="bcs", name=f"bcs{h}")
                        nc.vector.tensor_copy(bcs[:, :], bc[:, :])
                        nc.vector.tensor_mul(
                            headsT[h][:, :], acc[h][0:HD, :], bcs[:, :]
                        )

            # ================= Phase 4: output projection (transposed) =========
            with (
                tc.tile_pool(name="po", bufs=2, space="PSUM") as pop,
                tc.tile_pool(name="ot", bufs=2) as otp,
            ):
                for t in range(4):
                    po = pop.tile([128, 512], F32, tag="po")
                    for h in range(H):
                        nc.tensor.matmul(
                            po[:, :],
                            wo_sb[:, D * h + 128 * t : D * h + 128 * t + 128],
                            headsT[h][:, :],
                            start=(h == 0), stop=False,
                        )
                    nc.tensor.matmul(
                        po[:, :], wob[:, 128 * t : 128 * t + 128], ones[:, :],
                        start=False, stop=True,
                    )
                    ot = otp.tile([128, 512], F32, tag="ot")
                    nc.vector.tensor_copy(ot[:, :], po[:, :])
                    nc.sync.dma_start(out_d[128 * t : 128 * t + 128, :], ot[:, :])
            kvpool.__exit__(None, None, None)

    nc.compile()
    return nc


# ---------------------------------------------------------------------------
# Host-side sharding / assembly
# ---------------------------------------------------------------------------

def classify_mask(mask: np.ndarray) -> str:
    m = mask.reshape(S, S)
    r = np.arange(S)
    valid = r[None, :] <= r[:, None]
    if np.all(m[valid] == 0.0):
        if np.all(m[~valid] <= -1e8):
            return "causal"
    if np.all(m == 0.0):
        return "zeros"
    return "general"


def make_inputs(variant, c, query, value, mask, wq_k, wq_b, wkv_k, wkv_b, wo_k, wo_b):
    q = query.reshape(S, D)
    v = value.reshape(S, D)
    qa0 = QW * c
    qb0 = S - QW * (c + 1)
    qs = np.concatenate([q[qa0 : qa0 + QW], q[qb0 : qb0 + QW]], axis=0)
    vs = v[SHARD * c : SHARD * (c + 1)]

    f32 = np.float32
    biasA = np.zeros((128, 16), f32)
    biasB = np.zeros((128, 16), f32)
    tri = np.zeros((128, 2 * QW), f32)
    offs = np.zeros((1, 8), np.uint32)
    if variant == "causal":
        for g in range(16):
            if g >= c:
                biasA[:, g] = NEG
            if g > 14 - c:
                biasB[:, g] = NEG
        jj = np.arange(QW)[None, :]
        pp = np.arange(128)[:, None]
        tri[:, 0:QW] = (pp <= jj).astype(f32)
        tri[:, QW:] = (pp + 128 <= jj).astype(f32)
        uA = 2 * c
        uB = (S - QW * (c + 1)) // KU  # first diag unit of chunk B = 30-2c
        offs[0, 0:3] = (1024 * (uA // 4), 128 * (uA % 4),
                        1024 * (uA // 4) + 512 + 128 * (uA % 4))
        offs[0, 3:6] = (1024 * (uB // 4), 128 * (uB % 4),
                        1024 * (uB // 4) + 512 + 128 * (uB % 4))

    ins = {
        "qs": np.ascontiguousarray(qs, f32),
        "vs": np.ascontiguousarray(vs, f32),
        "wq": np.ascontiguousarray(wq_k, f32),
        "wkv": np.ascontiguousarray(wkv_k, f32),
        "wo": np.ascontiguousarray(wo_k, f32),
        "wqb": np.ascontiguousarray(wq_b.reshape(1, D), f32),
        "wkvb": np.ascontiguousarray(wkv_b.reshape(1, 2 * D), f32),
        "wob": np.ascontiguousarray(wo_b.reshape(1, D), f32),
        "biasA": biasA,
        "biasB": biasB,
        "trimask": tri,
        "offs": offs,
        "ident": np.eye(128, dtype=f32),
        "onesrow": np.ones((1, 512), f32),
    }
    if variant == "general":
        m = mask.reshape(S, S)
        rows = np.concatenate(
            [np.arange(qa0, qa0 + QW), np.arange(qb0, qb0 + QW)]
        )
        ins["expmT"] = np.ascontiguousarray(
            np.exp(np.minimum(m[rows, :], 80.0)).T, f32
        )
    return ins


def assemble(results):
    full = np.empty((S, D), np.float32)
    for c in range(NCORES):
        o = results[c]["outT"].T  # [512 q, 512 d]
        full[QW * c : QW * c + QW] = o[0:QW]
        full[S - QW * (c + 1) : S - QW * c] = o[QW:]
    return full.reshape(B, S, D)


_cache = {}
_runner_cache = {}
last_results = None


class _SpmdRunner:
    """Cached PJRT shard_map executor for a compiled Bass program (axon path).

    Mirrors bass2jax.run_bass_via_pjrt but keeps the jitted callable and
    device-put input buffers so repeat calls skip re-trace and re-transfer.
    """

    def __init__(self, nc):
        import jax
        from jax.sharding import Mesh, PartitionSpec, NamedSharding
        from jax.experimental.shard_map import shard_map
        import concourse.mybir as mb
        from concourse import bass2jax

        bass2jax.install_neuronx_cc_hook()
        self.nc = nc
        pname = nc.partition_id_tensor.name if nc.partition_id_tensor else None
        in_names, out_names, out_avals, zero_outs = [], [], [], []
        for alloc in nc.m.functions[0].allocations:
            if not isinstance(alloc, mb.MemoryLocationSet):
                continue
            name = alloc.memorylocations[0].name
            if alloc.kind == "ExternalInput":
                if name != pname:
                    in_names.append(name)
            elif alloc.kind == "ExternalOutput":
                shape = tuple(alloc.tensor_shape)
                dtype = mb.dt.np(alloc.dtype)
                out_names.append(name)
                out_avals.append(jax.core.ShapedArray(shape, dtype))
                zero_outs.append(np.zeros(shape, dtype))
        self.in_names, self.out_names = in_names, out_names
        self.out_avals, self.zero_outs = out_avals, zero_outs
        n_params, n_outs = len(in_names), len(out_names)
        all_names = in_names + out_names
        if pname is not None:
            all_names = all_names + [pname]

        def _body(*args):
            operands = list(args)
            if pname is not None:
                operands.append(bass2jax.partition_id_tensor())
            outs = bass2jax._bass_exec_p.bind(
                *operands,
                out_avals=tuple(out_avals),
                in_names=tuple(all_names),
                out_names=tuple(out_names),
                lowering_input_output_aliases=(),
                sim_require_finite=True,
                sim_require_nnan=True,
                nc=nc,
            )
            return tuple(outs)

        devices = jax.devices()[:NCORES]
        self.mesh = Mesh(np.asarray(devices), ("core",))
        self.spec = PartitionSpec("core")
        in_specs = (self.spec,) * (n_params + n_outs)
        out_specs = (self.spec,) * n_outs
        self.fn = jax.jit(
            shard_map(_body, mesh=self.mesh, in_specs=in_specs,
                      out_specs=out_specs, check_rep=False),
            donate_argnums=tuple(range(n_params, n_params + n_outs)),
            keep_unused=True,
        )
        self.sharding = NamedSharding(self.mesh, self.spec)
        self._jax = jax

    def concat_inputs(self, in_maps):
        return [
            np.concatenate([np.asarray(in_maps[c][n]) for c in range(NCORES)], axis=0)
            for n in self.in_names
        ]

    def put(self, concat_in):
        return [self._jax.device_put(a, self.sharding) for a in concat_in]

    def zeros(self):
        return [
            np.zeros((NCORES * z.shape[0], *z.shape[1:]), z.dtype)
            for z in self.zero_outs
        ]

    def __call__(self, bufs):
        jax = self._jax
        out = self.fn(*bufs, *self.zeros())
        out = jax.block_until_ready(out)
        return out

    def run(self, in_maps):
        out_arrs = self(self.put(self.concat_inputs(in_maps)))
        return [
            {
                n: np.asarray(out_arrs[i]).reshape(NCORES, *self.out_avals[i].shape)[c]
                for i, n in enumerate(self.out_names)
            }
            for c in range(NCORES)
        ]


def get_runner(variant):
    if variant not in _cache:
        _cache[variant] = build_bass(variant)
    if variant not in _runner_cache:
        _runner_cache[variant] = _SpmdRunner(_cache[variant])
    return _runner_cache[variant]


def kernel(query, value, mask, wq_k, wq_b, wkv_k, wkv_b, wo_k, wo_b, **run_kwargs):
    global last_results
    variant = classify_mask(np.asarray(mask))
    runner = get_runner(variant)
    in_maps = [
        make_inputs(variant, c, query, value, mask,
                    wq_k, wq_b, wkv_k, wkv_b, wo_k, wo_b)
        for c in range(NCORES)
    ]
    results = runner.run(in_maps)
    last_results = None
    return assemble(results)
